# revision 20
# baseline (speedup 1.0000x reference)
"""GroupHeadMLP Trainium2 kernel.

Model (eval): x[B, 8704] -> 32 block-diagonal heads (256->52->52->5, ELU)
over x[:, :8192] + one unique head (512->103->103->20, ELU) over
x[:, 8192:], concat -> [B, 180] -> dot with outW -> y[B].

Strategy: data-parallel over 8 NeuronCores (1024 rows each).

Host prep:
  - x cast to bf16 and transposed -> xT [8704, B] so features sit on
    SBUF partitions (contraction dim) with contiguous DMA.
  - Heads processed in pairs packed on psum partitions: group A at
    partitions 0-51, group B at 64-115 (64-offset so M=64 stationary
    blocks can target each half via tile_position).
  - No bias matmuls: biases ride the scalar-engine activation's
    per-partition bias operand.  Each ELU site computes
        e  = Exp(z + b)                       (ScalarE, bias free)
        zb = z + (b + 1)                      (Pool or DVE)
        h' = max(min(e, 1), zb)               (DVE STT, all-bf16 4x)
    which equals elu(z+b)+1.  The +1 is compensated in the next
    layer's folded bias (b - colsum(W)), and for the final dot by
    subtracting sum(outW) on the host.  Padding lanes carry exactly
    1.0 but multiply zero weight rows downstream.
  - Layer-3 outputs (M=10 per pair) are packed 4 pairs per PSUM tile
    via tile_position column strips; the final dot is 5 accumulating
    matmuls (K=128 x4 + K=32) into a [1, NT] PSUM.
"""

import sys

sys.path.insert(0, "/opt/trn_rl_repo")

import numpy as np

from concourse import bass, mybir, tile
from concourse.alu_op_type import AluOpType
from concourse.bass_utils import run_bass_kernel_spmd
from concourse.vector_clock import ScopedClock

F16 = np.float16
F32 = np.float32

G, F, H, O = 32, 256, 52, 5
UF, UH, UO = 512, 103, 20
B = 8192
NCORES = 8
BC = B // NCORES          # 1024 rows per core
NT = 512                  # free-dim (batch) tile; 2 tiles per core
NPAIR = G // 2            # 16 group pairs
NBT = BC // NT            # batch tiles per core

AF = mybir.ActivationFunctionType

# smalls tile column layout (bf16): w2 | w3 | uw1 | uw2 | uw3 | wout
SM_W2 = 0
SM_W3 = SM_W2 + 16 * 128
SM_UW1 = SM_W3 + 16 * 32
SM_UW2 = SM_UW1 + 4 * 128
SM_UW3 = SM_UW2 + 128
SM_WOUT = SM_UW3 + 32
SM_COLS = SM_WOUT + 8

# bias tile column layout (f32): per column pairs (bc, bc+1)
#   L1 pair p -> cols 2p, 2p+1            (p = 0..15)
#   uL1       -> cols 32, 33
#   negone    -> col 34  (bias for Exp at ones-lane sites)
# L2/L3/unique-L2/L3 biases ride the pad-ones-lane folded into the
# weight pad rows (pads of h' tiles carry exactly 1.0).
BI_L1 = 0
BI_UL1 = 32
BI_NEG = 34
BI_COLS = 35


# ---------------------------------------------------------------------------
# Workaround for this container's walrus: the Drain instruction (TPB_CTRL
# encoding) rejects >1 semaphore wait.  Tile's kernel-tail drain attaches one
# wait per touched proc.  Split them onto single-wait carrier NOPs instead.
_patched = False


MAX_WAITS = 1  # walrus in this container rejects >1 sem wait per instruction


def _apply_tile_patch():
    global _patched
    if _patched:
        return
    _patched = True

    orig_commit = tile.TileContext._commit_instruction

    def _commit_split_waits(self, inst, lazy_reg_writes=True):
        si = inst.sync_info
        if (
            si is not None
            and si.on_wait
            and len(si.on_wait) > MAX_WAITS
            and inst.engine != mybir.EngineType.Unassigned
        ):
            waits = list(si.on_wait)
            keep = waits[:MAX_WAITS]
            extra = waits[MAX_WAITS:]
            for w in extra:
                nop = mybir.InstNoOp(
                    name=self.nc.get_next_instruction_name(),
                    engine=inst.engine,
                    sync_info=mybir.SyncInfo(on_wait=[w], on_update=[]),
                    bass_nofuse=True,
                    ins=[],
                    outs=[],
                )
                orig_commit(self, nop, lazy_reg_writes=False)
            inst.sync_info = mybir.SyncInfo(on_wait=keep, on_update=si.on_update)
        return orig_commit(self, inst, lazy_reg_writes)

    tile.TileContext._commit_instruction = _commit_split_waits

    def _split_drain_and_barrier(self, tick_clock, wait_clock):
        vclock = tick_clock.global_clock
        for proc in range(len(vclock)):
            t = vclock[proc]
            if t > 0:
                nop = self.nc.sync.nop()
                req = ScopedClock()
                req.require_at_least(None, proc, t)
                wait_clock.add_sem_waits(nop.ins, req)
        self.nc.sync.drain()
        self.nc.all_engine_barrier()
        assert self.sems is not None
        popped = self.nc._tile_sem_poison_stack.pop()
        assert popped is self._sem_poison
        self.nc.clear_and_free_semaphores(list(self.sems.allocated().values()))
        self.nc.all_engine_barrier()

    tile.TileContext._drain_and_barrier = _split_drain_and_barrier


# ---------------------------------------------------------------------------
_NC_CACHE = None


def _build_program():
    global _NC_CACHE
    if _NC_CACHE is not None:
        return _NC_CACHE
    _apply_tile_patch()

    nc = bass.Bass("TRN2", target_bir_lowering=False, num_devices=NCORES)
    bf = mybir.dt.float16
    f32 = mybir.dt.float32

    xt = nc.dram_tensor("xt", [G * F + UF, BC], bf, kind="ExternalInput")
    w1 = nc.dram_tensor("w1", [128, 64 * 64], bf, kind="ExternalInput")
    smalls_d = nc.dram_tensor("smalls", [128, SM_COLS], bf, kind="ExternalInput")
    bias_d = nc.dram_tensor("biases", [128, BI_COLS], f32, kind="ExternalInput")
    y = nc.dram_tensor("y", [1, BC], bf, kind="ExternalOutput")

    with tile.TileContext(nc) as tc:
        with (
            tc.tile_pool(name="wpool", bufs=1) as wpool,
            tc.tile_pool(name="xpool", bufs=1) as xpool,
            tc.tile_pool(name="epool", bufs=6) as epool,
            tc.tile_pool(name="zpool", bufs=6) as zpool,
            tc.tile_pool(name="hpool", bufs=6) as hpool,
            tc.tile_pool(name="osb", bufs=1) as osb_pool,
            tc.tile_pool(name="ps1", bufs=3, space="PSUM") as ps1,
            tc.tile_pool(name="ps2", bufs=2, space="PSUM") as ps2,
            tc.tile_pool(name="ps3", bufs=2, space="PSUM") as ps3,
            tc.tile_pool(name="pso", bufs=1, space="PSUM") as pso,
        ):
            # ---------------- DMA emission (all upfront, x leads) ---------
            bsb = wpool.tile([128, BI_COLS], f32, name="biassb")
            nc.sync.dma_start(bsb[:], bias_d[:, :])

            smsb = wpool.tile([128, SM_COLS], bf, name="smallsb")
            w1tiles = {}
            xtiles = {}
            xus = {}

            def emit_w1(s, n):
                t_ = wpool.tile([128, n * 4 * 64], bf, tag=f"w1_{s}", name=f"w1sb_{s}")
                nc.sync.dma_start(t_[:], w1[:, s * 256: (s + n) * 256])
                for pp in range(s, s + n):
                    w1tiles[pp] = (t_, pp - s)

            def emit_x(nt, cs, n):
                col = slice(nt * NT, (nt + 1) * NT)
                xc = xpool.tile([128, 4 * n, NT], bf, tag=f"x{nt}_{cs}",
                                name=f"xc_{nt}_{cs}")
                src = xt[cs * 512: (cs + n) * 512, col]
                src = src.rearrange("(k pi) n -> pi k n", pi=128)
                nc.sync.dma_start(xc[:], src)
                for pp in range(cs, cs + n):
                    xtiles[nt, pp] = (xc, pp - cs)

            def emit_xu(nt):
                col = slice(nt * NT, (nt + 1) * NT)
                xu = xpool.tile([128, 4, NT], bf, tag=f"xu{nt}", name=f"xu_{nt}")
                src = xt[G * F: G * F + UF, col]
                src = src.rearrange("(k pi) n -> pi k n", pi=128)
                nc.sync.dma_start(xu[:], src)
                xus[nt] = xu

            emit_w1(0, 1)
            emit_x(0, 0, 1)
            emit_x(0, 1, 1)
            emit_w1(1, 1)
            nc.sync.dma_start(smsb[:, :SM_UW1], smalls_d[:, :SM_UW1])
            emit_x(0, 2, 2)
            emit_w1(2, 2)
            emit_x(0, 4, 2)
            emit_w1(4, 4)
            emit_x(0, 6, 2)
            emit_x(0, 8, 2)
            emit_w1(8, 4)
            emit_x(0, 10, 2)
            emit_x(0, 12, 2)
            emit_w1(12, 4)
            emit_x(0, 14, 2)
            nc.sync.dma_start(smsb[:, SM_UW1:], smalls_d[:, SM_UW1:])
            emit_xu(0)
            emit_xu(1)
            for cs in range(0, 16, 2):
                emit_x(1, cs, 2)

            # ---------------- site helpers --------------------------------
            def elu_l1(psum_ap, bc_ap, cc_ap, tag):
                """elu(z+b)+1 from raw psum z (layer-1 sites).

                e = Exp(z + b)      ACT (bias rides the activation op)
                m = min(e, 1)       Pool (immediate-scalar op, SBUF only)
                h = max(z + c, m)   DVE (single fused psum read via STT)
                """
                e = epool.tile([128, NT], bf, tag="e" + tag)
                nc.scalar.activation(e[:], psum_ap, AF.Exp, bias=bc_ap)
                m = zpool.tile([128, NT], bf, tag="z" + tag)
                nc.gpsimd.tensor_scalar(m[:], e[:], 1.0, None, AluOpType.min)
                h = hpool.tile([128, NT], bf, tag="h" + tag)
                nc.vector.scalar_tensor_tensor(
                    h[:], psum_ap, cc_ap, m[:], AluOpType.add, AluOpType.max,
                )
                return h

            def elu_ol(psum_ap, tag, nparts=128):
                """elu(z+b)+1 where psum = z+b+1 (bias rode the pad-ones-lane
                folded into the weight pad rows).

                e = Exp(psum - 1)            ACT
                h = max(min(e, 1), psum)     DVE STT
                """
                e = epool.tile([128, NT], bf, tag="e" + tag)
                nc.scalar.activation(
                    e[:nparts, :], psum_ap, AF.Exp,
                    bias=bsb[:nparts, BI_NEG: BI_NEG + 1],
                )
                h = hpool.tile([128, NT], bf, tag="h" + tag)
                nc.vector.scalar_tensor_tensor(
                    h[:nparts, :], e[:nparts, :], 1.0, psum_ap,
                    AluOpType.min, AluOpType.max,
                )
                return h

            def w2s(p):
                return smsb[:, SM_W2 + p * 128: SM_W2 + (p + 1) * 128]

            def w3s(p):
                return smsb[:, SM_W3 + p * 32: SM_W3 + (p + 1) * 32]

            NU = NPAIR  # unique-head pseudo-pair index
            osb = osb_pool.tile([1, 2 * NT], bf, name="osb")

            # ---------------- software-pipelined tile bodies --------------
            # slot s emits: L1 mm (v=order[s]) | site1 (order[s-1]) |
            #   L2 mm (order[s-2]) | site2 (order[s-3]) | L3 mm (order[s-4])
            # pair-quad t site3/out and the unique site3/out trail by the
            # appropriate skew.  Tile 1 runs the unique head FIRST so the
            # serial u-chain is not the kernel drain tail.
            for nt in range(NBT):
                col = slice(nt * NT, (nt + 1) * NT)
                outp = pso.tile([1, NT], f32, tag="outp", name=f"outp_{nt}")
                xu = xus[nt]
                order = (list(range(NPAIR)) + [NU]) if nt == 0 else \
                    ([NU] + list(range(NPAIR)))
                # slot index at which pair p's L1 mm is emitted
                sl_of = {v: i for i, v in enumerate(order)}
                # pair-quad t's last L3 mm is at slot sl_of[4t+3]+4
                quad_done = {t: sl_of[4 * t + 3] + 4 for t in range(4)}
                u_done = sl_of[NU] + 4
                # out-mm accumulation order: whichever site3 completes first
                outs = sorted(
                    [("q", t, quad_done[t] + 1) for t in range(4)]
                    + [("u", None, u_done + 1)],
                    key=lambda z: z[2],
                )
                out_emit = {}
                for oi, (kind, t, s3slot) in enumerate(outs):
                    out_emit.setdefault(s3slot + 1, []).append(
                        (kind, t, oi == 0, oi == len(outs) - 1)
                    )
                site3_emit = {}
                for kind, t, s3slot in outs:
                    site3_emit.setdefault(s3slot, []).append((kind, t))

                ps1t = {}
                ps2t = {}
                ps3t = {}
                h1t = {}
                h2t = {}
                f3st = {}
                nslots = len(order) + 8

                for slot in range(nslots):
                    # S0: layer-1 matmuls
                    if slot < len(order):
                        v = order[slot]
                        if v == NU:
                            u1p = ps1.tile([128, NT], f32, tag="ps1",
                                           name=f"u1p_{nt}")
                            ps1t[NU] = u1p
                            for k in range(4):
                                nc.tensor.matmul(
                                    u1p[:],
                                    smsb[:, SM_UW1 + k * 128: SM_UW1 + (k + 1) * 128],
                                    xu[:, k: k + 1, :],
                                    start=(k == 0), stop=(k == 3),
                                )
                        else:
                            xa, loc = xtiles[nt, v]
                            w1t, wloc = w1tiles[v]
                            h1p = ps1.tile([128, NT], f32, tag="ps1",
                                           name=f"h1p_{nt}_{v}")
                            ps1t[v] = h1p
                            for k in range(4):
                                half = 64 * (k // 2)
                                nc.tensor.matmul(
                                    h1p[half: half + 64, :],
                                    w1t[:, (4 * wloc + k) * 64: (4 * wloc + k + 1) * 64],
                                    xa[:, 4 * loc + k: 4 * loc + k + 1, :],
                                    start=(k % 2 == 0), stop=(k % 2 == 1),
                                    tile_position=(0, half),
                                )

                    # S3: site2 first (its inputs are the oldest / readiest)
                    if 0 <= slot - 3 < len(order):
                        v = order[slot - 3]
                        h2t[v] = elu_ol(ps2t.pop(v)[:], "2")

                    # S1: site1
                    if 0 <= slot - 1 < len(order):
                        v = order[slot - 1]
                        if v == NU:
                            h1t[NU] = elu_l1(
                                ps1t.pop(NU)[:],
                                bsb[:, BI_UL1: BI_UL1 + 1],
                                bsb[:, BI_UL1 + 1: BI_UL1 + 2],
                                "1",
                            )
                        else:
                            h1t[v] = elu_l1(
                                ps1t.pop(v)[:],
                                bsb[:, BI_L1 + 2 * v: BI_L1 + 2 * v + 1],
                                bsb[:, BI_L1 + 2 * v + 1: BI_L1 + 2 * v + 2],
                                "1",
                            )

                    # S2: layer-2 matmul
                    if 0 <= slot - 2 < len(order):
                        v = order[slot - 2]
                        h2p = ps2.tile([128, NT], f32, tag="ps2",
                                       name=f"h2p_{nt}_{v}")
                        ps2t[v] = h2p
                        lhs = smsb[:, SM_UW2: SM_UW2 + 128] if v == NU else w2s(v)
                        nc.tensor.matmul(
                            h2p[:], lhs, h1t.pop(v)[:], start=True, stop=True,
                        )

                    # S4: layer-3 matmul
                    if 0 <= slot - 4 < len(order):
                        v = order[slot - 4]
                        if v == NU:
                            u3p = ps3.tile([128, NT], f32, tag="ps3",
                                           name=f"u3p_{nt}")
                            ps3t["u"] = u3p
                            nc.tensor.matmul(
                                u3p[0:32, :], smsb[:, SM_UW3: SM_UW3 + 32],
                                h2t.pop(NU)[:],
                                start=True, stop=True, tile_position=(0, 0),
                            )
                        else:
                            t, j = divmod(v, 4)
                            if j == 0:
                                ps3t[t] = ps3.tile([128, NT], f32, tag="ps3",
                                                   name=f"f3p_{nt}_{t}")
                            nc.tensor.matmul(
                                ps3t[t][32 * j: 32 * j + 32, :],
                                w3s(v),
                                h2t.pop(v)[:],
                                start=True, stop=True,
                                tile_position=(0, 32 * j),
                            )

                    # S6: site3
                    for kind, t in site3_emit.get(slot, []):
                        if kind == "u":
                            f3st["u"] = elu_ol(ps3t.pop("u")[0:32, :], "3",
                                               nparts=32)
                        else:
                            f3st[t] = elu_ol(ps3t.pop(t)[:], "3")

                    # S7: out matmuls (accumulation order = emission order)
                    for kind, t, first, last in out_emit.get(slot, []):
                        if kind == "u":
                            nc.tensor.matmul(
                                outp[:],
                                smsb[0:32, SM_WOUT + 4: SM_WOUT + 5],
                                f3st.pop("u")[0:32, :],
                                start=first, stop=last,
                                skip_group_check=True,
                            )
                        else:
                            nc.tensor.matmul(
                                outp[:],
                                smsb[:, SM_WOUT + t: SM_WOUT + t + 1],
                                f3st.pop(t)[:],
                                start=first, stop=last,
                                skip_group_check=True,
                            )

                    if slot == nslots - 1:
                        nc.scalar.activation(
                            osb[0:1, nt * NT: (nt + 1) * NT], outp[:], AF.Copy
                        )
                        if nt == NBT - 1:
                            nc.sync.dma_start(y[0:1, :], osb[:])

    _NC_CACHE = nc
    return nc


# ---------------------------------------------------------------------------
_WEIGHTS_CACHE = None


def _pack_weights(W1, b1, W2, b2, W3, b3, uW1, ub1, uW2, ub2, uW3, ub3, outW):
    # A group at psum partitions 0-51, B group at 64-115
    w1h = np.zeros((128, 64, 64), F32)
    for p in range(NPAIR):
        for k in range(4):
            g = 2 * p + (k // 2)
            fo = 128 * (k % 2)
            w1h[:, 4 * p + k, 0:H] = W1[g, fo: fo + 128, :]

    sm = np.zeros((128, SM_COLS), F32)
    bih = np.zeros((128, BI_COLS), F32)
    bih[:, BI_NEG] = -1.0

    for p in range(NPAIR):
        # L2/L3 biases (+1) ride row H (a pad lane of h', which carries
        # exactly 1.0); pad columns get a 1.0 there to self-sustain.
        blk2 = sm[:, SM_W2 + p * 128: SM_W2 + (p + 1) * 128]
        blk2[0:H, 0:H] = W2[2 * p]
        blk2[64: 64 + H, 64: 64 + H] = W2[2 * p + 1]
        blk2[H, 0:H] = b2[2 * p] - W2[2 * p].sum(axis=0) + 1.0
        blk2[H, 64: 64 + H] = b2[2 * p + 1] - W2[2 * p + 1].sum(axis=0) + 1.0
        blk2[H, H: 64] = 1.0
        blk2[H, 64 + H:] = 1.0

        blk3 = sm[:, SM_W3 + p * 32: SM_W3 + (p + 1) * 32]
        blk3[0:H, 0:O] = W3[2 * p]
        blk3[64: 64 + H, O: 2 * O] = W3[2 * p + 1]
        blk3[H, 0:O] = b3[2 * p] - W3[2 * p].sum(axis=0) + 1.0
        blk3[H, O: 2 * O] = b3[2 * p + 1] - W3[2 * p + 1].sum(axis=0) + 1.0

        c = bih[:, BI_L1 + 2 * p]
        c[0:H] = b1[2 * p]
        c[64: 64 + H] = b1[2 * p + 1]
        bih[:, BI_L1 + 2 * p + 1] = c + 1.0

    for k in range(4):
        sm[:, SM_UW1 + k * 128: SM_UW1 + k * 128 + UH] = uW1[128 * k: 128 * (k + 1), :]
    sm[0:UH, SM_UW2: SM_UW2 + UH] = uW2
    sm[UH, SM_UW2: SM_UW2 + UH] = ub2 - uW2.sum(axis=0) + 1.0
    sm[UH, SM_UW2 + UH: SM_UW2 + 128] = 1.0
    sm[0:UH, SM_UW3: SM_UW3 + UO] = uW3
    sm[UH, SM_UW3: SM_UW3 + UO] = ub3 - uW3.sum(axis=0) + 1.0

    c = bih[:, BI_UL1]
    c[0:UH] = ub1
    bih[:, BI_UL1 + 1] = c + 1.0

    for t in range(4):
        for j in range(4):
            p = 4 * t + j
            sm[32 * j: 32 * j + 2 * O, SM_WOUT + t] = outW[10 * p: 10 * p + 10]
    sm[0:UO, SM_WOUT + 4] = outW[G * O:]

    return {
        "w1": np.ascontiguousarray(w1h.reshape(128, 64 * 64)).astype(F16),
        "smalls": sm.astype(F16),
        "biases": bih.astype(F32),
    }, float(outW.sum())


def kernel(x, W1, b1, W2, b2, W3, b3, uW1, ub1, uW2, ub2, uW3, ub3, outW):
    global _WEIGHTS_CACHE
    x = np.asarray(x, F32)
    nc = _build_program()

    if _WEIGHTS_CACHE is None:
        _WEIGHTS_CACHE = _pack_weights(
            np.asarray(W1, F32), np.asarray(b1, F32),
            np.asarray(W2, F32), np.asarray(b2, F32),
            np.asarray(W3, F32), np.asarray(b3, F32),
            np.asarray(uW1, F32), np.asarray(ub1, F32),
            np.asarray(uW2, F32), np.asarray(ub2, F32),
            np.asarray(uW3, F32), np.asarray(ub3, F32),
            np.asarray(outW, F32),
        )
    wmap, c0 = _WEIGHTS_CACHE

    xtr = np.ascontiguousarray(x.astype(F16).T)  # [8704, B]
    in_maps = []
    for c in range(NCORES):
        m = dict(wmap)
        m["xt"] = np.ascontiguousarray(xtr[:, c * BC: (c + 1) * BC])
        in_maps.append(m)

    res = run_bass_kernel_spmd(nc, in_maps, list(range(NCORES)))
    out = np.empty(B, F32)
    for c in range(NCORES):
        out[c * BC: (c + 1) * BC] = res.results[c]["y"][0].astype(F32) - c0
    return out
